# revision 40
# baseline (speedup 1.0000x reference)
"""Multi-head linear attention on 8 Trainium2 NeuronCores.

Sharding: data-parallel over batch (4) x tensor-parallel over heads (2 groups
of 8). Core c handles batch c//2, head-group c%2. Each core computes its
head-group's partial output projection; the host sums the two partials per
batch.

Per-core math (F=1024, L=8192, HG=8 heads, D=64, HD=512):
  k = xkv @ Wk ; v = xkv @ Wv          (natural orientation, [t, hd])
  kp = phi(k) = exp(min(k,0)) + max(k,0)
  state_h = kp_h^T @ [v_h | 1]         ([d, e] kv-state + ksum column)
  qT = Wq^T @ xq^T ; qp = phi(q)       (transposed orientation, [hd, t])
  denom = Blk^T @ qp^T  (Blk = block-diag ksum), z = 1/(denom+eps)
  numT_h = kv_h^T @ qp_h^T ; outT = numT * bcast(z)
  yT = Wo^T @ outT                     ([f, t] partial, summed on host)
"""

import sys

sys.path.insert(0, "/opt/trn_rl_repo")

import numpy as np

import concourse.bass as bass  # noqa: F401  (import keeps bass registered)
import concourse.tile as tile
from concourse import bacc, mybir
from concourse.bass_utils import run_bass_kernel_spmd

F32 = mybir.dt.float32
F32R = mybir.dt.float32r
BF16 = mybir.dt.bfloat16
AF = mybir.ActivationFunctionType
ALU = mybir.AluOpType

B, L_FULL, F = 4, 8192, 1024
H, D = 16, 64
N_CORES = 8
HG = H // 2  # heads per core = 8
HD = HG * D  # 512
EPS = 1e-6


def build_nc(L=L_FULL, TQ=512, TK=128):
    NKT = L // TK
    NQT = L // TQ
    FA = F // 128  # 8 f-tiles
    NM = HD // 128  # 4 hd-tiles

    nc = bacc.Bacc("TRN2", target_bir_lowering=False, debug=False)

    xqT = nc.dram_tensor("xqT", [F, L], F32R, kind="ExternalInput")
    xkvT = nc.dram_tensor("xkvT", [F, L], F32R, kind="ExternalInput")
    wq = nc.dram_tensor("wq", [F, HD], F32R, kind="ExternalInput")
    wk = nc.dram_tensor("wk", [F, HD], F32R, kind="ExternalInput")
    wv = nc.dram_tensor("wv", [F, HD], F32R, kind="ExternalInput")
    wo = nc.dram_tensor("wo", [HD, F], F32R, kind="ExternalInput")
    em = nc.dram_tensor("ematrix", [HG, HD // 128, 128], F32R, kind="ExternalInput")
    yT = nc.dram_tensor("yT", [F, L], F32, kind="ExternalOutput")

    xqT_r = xqT.rearrange("(a p) l -> p a l", p=128)
    xkvT_r = xkvT.rearrange("(a p) l -> p a l", p=128)
    wq_r = wq.rearrange("(a p) n -> p a n", p=128)
    wk_r = wk.rearrange("(a p) n -> p a n", p=128)
    wv_r = wv.rearrange("(a p) n -> p a n", p=128)
    wo_r = wo.rearrange("(m p) f -> p m f", p=128)
    yT_r = yT.rearrange("(a p) l -> p a l", p=128)

    with tile.TileContext(nc) as tc:
        with (
            tc.tile_pool(name="singles", bufs=1) as singles,
            tc.tile_pool(name="kv_in", bufs=3) as kv_in,
            tc.tile_pool(name="kwork", bufs=2) as kwork,
            tc.tile_pool(name="q_in", bufs=2) as q_in,
            tc.tile_pool(name="qwork", bufs=2) as qwork,
            tc.tile_pool(name="yout", bufs=2) as yout,
        ):
            ps_state_ctx = tc.tile_pool(name="ps_state", bufs=1, space="PSUM")
            ps_state = ps_state_ctx.__enter__()
            # ---- weights (chunked DMAs so the first matmuls start early;
            # wk/wv first since phase 1 needs them) ----
            wq_sb = singles.tile([128, FA, HD], F32R)
            wk_sb = singles.tile([128, FA, HD], F32R)
            wv_sb = singles.tile([128, FA, HD], F32R)
            wo_sb = singles.tile([128, NM, F], F32R)
            for a in range(FA):
                nc.scalar.dma_start(out=wk_sb[:, a, :], in_=wk_r[:, a, :])
            for a in range(FA):
                nc.scalar.dma_start(out=wv_sb[:, a, :], in_=wv_r[:, a, :])
            # persistent bf16 ones for the ksum column of the state matmul
            ones_sb = singles.tile([128, NM, 1], BF16)
            nc.vector.memset(ones_sb[:], 1.0)

            # persistent state accumulators, one PSUM bank per head pair j.
            # Single matmul per pair: lhsT = kp[:, pair d-range] (128 wide),
            # rhs = [v_h0 | v_h1 | 1] (129 wide). Rows 0:64 x cols 0:64 give
            # head 2j's kv, rows 64:128 x cols 64:128 head 2j+1's kv (cross
            # blocks never read); col 128 is the stacked ksum pair.
            st_ps = [
                ps_state.tile([128, 2 * D + 1], F32, tag=f"st{j}", name=f"st_ps{j}")
                for j in range(NM)
            ]

            # ---- phase 1: keys/values ----
            ps_kv_ctx = tc.tile_pool(name="ps_kv", bufs=2, space="PSUM")
            ps_kv = ps_kv_ctx.__enter__()
            xq_pre = {}
            for kt in range(NKT):
                if kt == NKT // 2:
                    # queries-side weights, needed only in phase 2
                    for a in range(FA):
                        nc.scalar.dma_start(out=wq_sb[:, a, :], in_=wq_r[:, a, :])
                    for m in range(NM):
                        nc.scalar.dma_start(out=wo_sb[:, m, :], in_=wo_r[:, m, :])
                if NKT > 16 and kt in (NKT - 8, NKT - 4):
                    qi = 0 if kt == NKT - 8 else 1
                    t_pre = q_in.tile([128, FA, TQ], F32R, tag="xq", name=f"xq_pre{qi}")
                    nc.sync.dma_start(
                        out=t_pre[:], in_=xqT_r[:, :, qi * TQ : (qi + 1) * TQ]
                    )
                    xq_pre[qi] = t_pre
                xkv_t = kv_in.tile([128, FA, TK], F32R)
                nc.sync.dma_start(
                    out=xkv_t[:], in_=xkvT_r[:, :, kt * TK : (kt + 1) * TK]
                )
                pk = ps_kv.tile([128, HD], F32, tag="pk")
                pv = ps_kv.tile([128, HD], F32, tag="pv")
                for a in range(FA):
                    nc.tensor.matmul(
                        pk[:],
                        lhsT=xkv_t[:, a, :],
                        rhs=wk_sb[:, a, :],
                        start=(a == 0),
                        stop=(a == FA - 1),
                    )
                for a in range(FA):
                    nc.tensor.matmul(
                        pv[:],
                        lhsT=xkv_t[:, a, :],
                        rhs=wv_sb[:, a, :],
                        start=(a == 0),
                        stop=(a == FA - 1),
                    )
                # phi(k) = exp(min(k,0)) + max(k,0); kp/v in bf16 (state-only)
                tmp = kwork.tile([128, HD], F32, tag="tmp")
                nc.vector.tensor_scalar_min(tmp[:], pk[:], 0.0)
                ek = kwork.tile([128, HD], F32, tag="ek")
                nc.scalar.activation(ek[:], tmp[:], AF.Exp)
                kp = kwork.tile([128, HD], BF16, tag="kp")
                nc.vector.scalar_tensor_tensor(
                    kp[:], in0=pk[:], scalar=0.0, in1=ek[:], op0=ALU.max, op1=ALU.add
                )
                # v pairs with trailing ones column: [v_h0 | v_h1 | 1]
                v_sb = kwork.tile([128, NM, 2 * D + 1], BF16, tag="v")
                nc.scalar.copy(
                    out=v_sb[:, :, 0 : 2 * D],
                    in_=pv[:].rearrange("p (j w) -> p j w", j=NM),
                )
                nc.vector.tensor_copy(v_sb[:, :, 2 * D : 2 * D + 1], ones_sb[:])
                # state accumulation, one stream per head pair bank
                for j in range(NM):
                    nc.tensor.matmul(
                        st_ps[j][:],
                        lhsT=kp[:, 2 * j * D : (2 * j + 2) * D],
                        rhs=v_sb[:, j, :],
                        start=(kt == 0),
                        stop=(kt == NKT - 1),
                    )

            ps_kv_ctx.__exit__(None, None, None)

            # ---- state to SBUF: block-diagonal kv pairs + block-diag ksum.
            # fp32r matmuls cannot write PSUM at partition offsets, so the
            # per-pair numerator/denominator use block-diagonal stationaries
            # (zero off-blocks) and plain partition-0 outputs instead.
            kv2 = singles.tile([128, NM, 128], F32R)
            nc.vector.memset(kv2[:].bitcast(F32), 0.0)
            blk = singles.tile([128, NM, HG], F32R)
            nc.vector.memset(blk[:].bitcast(F32), 0.0)
            for j in range(NM):
                nc.vector.tensor_copy(kv2[0:64, j, 0:D], st_ps[j][0:64, 0:D])
                nc.vector.tensor_copy(
                    kv2[64:128, j, D:128], st_ps[j][64:128, D : 2 * D]
                )
                nc.vector.tensor_copy(
                    blk[0:64, j, 2 * j : 2 * j + 1], st_ps[j][0:64, 2 * D : 2 * D + 1]
                )
                nc.vector.tensor_copy(
                    blk[64:128, j, 2 * j + 1 : 2 * j + 2],
                    st_ps[j][64:128, 2 * D : 2 * D + 1],
                )
            ps_state_ctx.__exit__(None, None, None)

            # broadcast matrix E: E[2j, j, 0:64] = 1, E[2j+1, j, 64:128] = 1
            e_sb = singles.tile([HG, NM, 128], F32R)
            nc.sync.dma_start(out=e_sb[:], in_=em[:])

            # ---- phase 2: queries ----
            # PSUM budget (8 banks): pq 2 + pd 1 + att(pzb,pn) 3 + py 2
            ps_q_ctx = tc.tile_pool(name="ps_q", bufs=3, space="PSUM")
            ps_q = ps_q_ctx.__enter__()
            ps_att_ctx = tc.tile_pool(name="ps_att", bufs=3, space="PSUM")
            ps_att = ps_att_ctx.__enter__()
            ps_y_ctx = tc.tile_pool(name="ps_y", bufs=2, space="PSUM")
            ps_y = ps_y_ctx.__enter__()
            for qt in range(NQT):
                if qt in xq_pre:
                    xq_t = xq_pre.pop(qt)
                else:
                    xq_t = q_in.tile([128, FA, TQ], F32R, tag="xq", name=f"xq_t{qt}")
                    nc.sync.dma_start(
                        out=xq_t[:], in_=xqT_r[:, :, qt * TQ : (qt + 1) * TQ]
                    )
                qp = qwork.tile([128, NM, TQ], F32R, tag="qp")
                for m in range(NM):
                    pq = ps_q.tile([128, TQ], F32, tag="pq")
                    for a in range(FA):
                        nc.tensor.matmul(
                            pq[:],
                            lhsT=wq_sb[:, a, m * 128 : (m + 1) * 128],
                            rhs=xq_t[:, a, :],
                            start=(a == 0),
                            stop=(a == FA - 1),
                        )
                    tmp2 = qwork.tile([128, TQ], F32, tag="tmp2")
                    nc.vector.tensor_scalar_min(tmp2[:], pq[:], 0.0)
                    eq = qwork.tile([128, TQ], F32, tag="eq")
                    nc.scalar.activation(eq[:], tmp2[:], AF.Exp)
                    nc.vector.scalar_tensor_tensor(
                        qp[:, m, :],
                        in0=pq[:],
                        scalar=0.0,
                        in1=eq[:],
                        op0=ALU.max,
                        op1=ALU.add,
                    )
                # denominator [HG, TQ]
                pd = ps_att.tile([HG, TQ], F32, tag="att", name="pd")
                for m in range(NM):
                    nc.tensor.matmul(
                        pd[:],
                        lhsT=blk[:, m, :],
                        rhs=qp[:, m, :],
                        start=(m == 0),
                        stop=(m == NM - 1),
                    )
                z_sb = qwork.tile([HG, TQ], F32R, tag="z")
                HT = TQ // 2
                for u in range(2):
                    zs = z_sb[:, u * HT : (u + 1) * HT]
                    nc.vector.tensor_scalar_add(zs, pd[:, u * HT : (u + 1) * HT], EPS)
                    with nc.allow_low_precision(reason="z rounds to f32r"):
                        nc.vector.reciprocal(zs, zs)
                # numerator + z multiply
                outT = qwork.tile([128, NM, TQ], F32R, tag="outT")
                for m in range(NM):
                    pzb = ps_att.tile([128, TQ], F32, tag="att", name="pzb")
                    for u in range(2):
                        nc.tensor.matmul(
                            pzb[:, u * HT : (u + 1) * HT],
                            lhsT=e_sb[:, m, :],
                            rhs=z_sb[:, u * HT : (u + 1) * HT],
                            start=(u == 0),
                            stop=(u == 1),
                        )
                    zb_sb = qwork.tile([128, TQ], F32, tag="zb")
                    nc.scalar.copy(zb_sb[:], pzb[:])
                    pn = ps_att.tile([128, TQ], F32, tag="att", name="pn")
                    nc.tensor.matmul(
                        pn[:],
                        lhsT=kv2[:, m, :],
                        rhs=qp[:, m, :],
                        start=True,
                        stop=True,
                    )
                    nc.vector.tensor_mul(outT[:, m, :], pn[:], zb_sb[:])
                # output projection
                y_sb = yout.tile([128, FA, TQ], F32)
                for fo in range(FA):
                    py = ps_y.tile([128, TQ], F32, tag="py")
                    for m in range(NM):
                        nc.tensor.matmul(
                            py[:],
                            lhsT=wo_sb[:, m, fo * 128 : (fo + 1) * 128],
                            rhs=outT[:, m, :],
                            start=(m == 0),
                            stop=(m == NM - 1),
                        )
                    nc.scalar.copy(out=y_sb[:, fo, :], in_=py[:])
                nc.scalar.dma_start(
                    out=yT_r[:, 0 : FA // 2, qt * TQ : (qt + 1) * TQ],
                    in_=y_sb[:, 0 : FA // 2, :],
                )
                nc.scalar.dma_start(
                    out=yT_r[:, FA // 2 : FA, qt * TQ : (qt + 1) * TQ],
                    in_=y_sb[:, FA // 2 : FA, :],
                )
            ps_y_ctx.__exit__(None, None, None)
            ps_att_ctx.__exit__(None, None, None)
            ps_q_ctx.__exit__(None, None, None)

    nc.finalize()
    return nc


_NC_CACHE = {}


def _get_nc(L):
    if L not in _NC_CACHE:
        _NC_CACHE[L] = build_nc(L=L)
    return _NC_CACHE[L]


def make_in_maps(inputs_q, inputs_kv, Wq, Wk, Wv, Wo):
    inputs_q = np.asarray(inputs_q, dtype=np.float32)
    inputs_kv = np.asarray(inputs_kv, dtype=np.float32)
    Wq = np.asarray(Wq, dtype=np.float32)
    Wk = np.asarray(Wk, dtype=np.float32)
    Wv = np.asarray(Wv, dtype=np.float32)
    Wo = np.asarray(Wo, dtype=np.float32)
    b_ = inputs_q.shape[0]
    xqT = [np.ascontiguousarray(inputs_q[b].T) for b in range(b_)]
    xkvT = [np.ascontiguousarray(inputs_kv[b].T) for b in range(b_)]
    f_ = Wq.shape[0]
    wq_g = [
        np.ascontiguousarray(Wq[:, g * HG : (g + 1) * HG, :].reshape(f_, HD))
        for g in range(2)
    ]
    wk_g = [
        np.ascontiguousarray(Wk[:, g * HG : (g + 1) * HG, :].reshape(f_, HD))
        for g in range(2)
    ]
    wv_g = [
        np.ascontiguousarray(Wv[:, g * HG : (g + 1) * HG, :].reshape(f_, HD))
        for g in range(2)
    ]
    wo_g = [
        np.ascontiguousarray(Wo[g * HG : (g + 1) * HG].reshape(HD, f_))
        for g in range(2)
    ]
    em = make_ematrix()
    in_maps = []
    for c in range(2 * b_):
        b, g = c // 2, c % 2
        in_maps.append(
            {
                "xqT": xqT[b],
                "xkvT": xkvT[b],
                "wq": wq_g[g],
                "wk": wk_g[g],
                "wv": wv_g[g],
                "wo": wo_g[g],
                "ematrix": em,
            }
        )
    return in_maps



def make_ematrix():
    em = np.zeros((HG, HD // 128, 128), dtype=np.float32)
    for j in range(HD // 128):
        em[2 * j, j, 0:64] = 1.0
        em[2 * j + 1, j, 64:128] = 1.0
    return em


def run(inputs_q, inputs_kv, Wq, Wk, Wv, Wo, trace=False, **spmd_kwargs):
    l_ = np.asarray(inputs_q).shape[1]
    nc = _get_nc(l_)
    in_maps = make_in_maps(inputs_q, inputs_kv, Wq, Wk, Wv, Wo)
    res = run_bass_kernel_spmd(
        nc, in_maps, list(range(len(in_maps))), trace=trace, **spmd_kwargs
    )
    b_ = len(in_maps) // 2
    out = np.empty((b_, l_, F), dtype=np.float32)
    for b in range(b_):
        np.copyto(out[b], (res.results[2 * b]["yT"] + res.results[2 * b + 1]["yT"]).T)
    return out, res


def kernel(inputs_q, inputs_kv, Wq, Wk, Wv, Wo):
    out, _ = run(inputs_q, inputs_kv, Wq, Wk, Wv, Wo)
    return out
